# revision 3
# baseline (speedup 1.0000x reference)
"""ClusterLoss (vq codebook coverage entropy) Trainium2 kernel.

Problem (hardcoded shapes):
    selected_frames [B=512, K=64, D=512] f32, cluster_centers [N=1024, D=512] f32.
    assignments = argmin_n ||x_{b,k} - c_n||^2
    coverage[n]  = sum_b 1[any_k assignments[b,k] == n]
    prob = coverage / (B*K);  out = -sum prob*log(prob+1e-10)   (scalar f32)

Strategy:
    * Data-parallel over batch: 64 batch elements (4096 frames) per core on 8
      cores; cluster_centers replicated.
    * argmin_n dist^2 == argmax_n s where s = cross - 0.5*||c_n||^2 (the x^2
      term is constant per row).  Per 128-frame tile: s is computed with bf16
      matmuls accumulating in fp32 PSUM; the -0.5*||c||^2 bias is folded in as
      a rank-2 (hi+lo bf16) matmul so it keeps ~fp32 accuracy.
    * mask = sign(s - rowmax) in {-1, 0} (ScalarE activation, per-partition
      bias = -rowmax from a VectorE reduce).  0 marks the argmax column(s).
    * count[b, n] = sum_k mask  via matmul with a block-ones stationary
      matrix (maps the tile's 2x64 frame rows to 2 batch rows, accumulated
      across all 32 tiles in a persistent PSUM accumulator).
      count = A - 64 where A = #argmax hits, so coverage = 1[count >= -63.5].
    * Final 64->1 partition reduction via a ones matmul; host sums the 8
      per-core [1024] histograms and computes the entropy.

bf16 effect on assignments was measured off-line: ~100/32768 flipped
assignments, |d entropy| ~ 1e-4 absolute (2e-5 relative).
"""

import os
import numpy as np
import ml_dtypes

import concourse.bass as bass
import concourse.mybir as mybir
import concourse.tile as tile
from concourse import bacc
from concourse.bass_utils import run_bass_kernel_spmd

BF16 = ml_dtypes.bfloat16

B, K, D, N = 512, 64, 512, 1024
NCORES = 8
B_PER_CORE = B // NCORES          # 64
F_PER_CORE = B_PER_CORE * K       # 4096 frames
NT = F_PER_CORE // 128            # 32 tiles of 128 frames (2 batch elems)
GROUP = 8                         # f-tiles per input DMA (1 MiB transfers)
COUNT_DELAY = 3                   # tiles to delay the count matmul (pipelining)

_CACHE = {}
LAST_RESULTS = None
RUN_KWARGS = {}  # extra kwargs for run_bass_kernel_spmd (profiling harness hook)


def _build_nc():
    nc = bacc.Bacc("TRN2", target_bir_lowering=False, debug=False,
                   num_devices=NCORES)
    dt = mybir.dt

    # DRAM I/O (per core).  fT layout: [d, t*512 + c*128 + f] where
    # fT[d, t, c, f] = frames[128*t + f, 128*c + d]  (frames pre-transposed on
    # host so every matmul operand is already D-major).
    fT = nc.dram_tensor("fT", [128, NT * 512], dt.bfloat16, kind="ExternalInput")
    cT = nc.dram_tensor("cT", [4, 128, N], dt.bfloat16, kind="ExternalInput")
    csq2 = nc.dram_tensor("csq2", [2, N], dt.bfloat16, kind="ExternalInput")
    bmap = nc.dram_tensor("bmap", [128, 127], dt.bfloat16, kind="ExternalInput")
    cov = nc.dram_tensor("cov", [1, N], dt.float32, kind="ExternalOutput")

    with tile.TileContext(nc) as tc:
        with (
            tc.tile_pool(name="const", bufs=1) as cpool,
            tc.tile_pool(name="frames", bufs=2) as fpool,
            tc.tile_pool(name="mask", bufs=COUNT_DELAY + 2) as mpool,
            tc.tile_pool(name="mneg", bufs=4) as mnpool,
            tc.tile_pool(name="spsum", bufs=2, space="PSUM") as spool,
            tc.tile_pool(name="cpsum", bufs=1, space="PSUM") as kpool,
            tc.tile_pool(name="outp", bufs=1) as opool,
        ):
            # --- constants ---
            ct = []
            for c in range(4):
                t_ = cpool.tile([128, N], dt.bfloat16, tag=f"ct{c}", name=f"ct{c}")
                nc.sync.dma_start(out=t_[:], in_=cT[c])
                ct.append(t_)
            csq2_t = cpool.tile([2, N], dt.bfloat16, tag="csq2", name="csq2_t")
            nc.sync.dma_start(out=csq2_t[:], in_=csq2[:])
            bmap_t = cpool.tile([128, 127], dt.bfloat16, tag="bmap", name="bmap_t")
            nc.sync.dma_start(out=bmap_t[:], in_=bmap[:])
            ones2_t = cpool.tile([2, 128], dt.bfloat16, tag="ones2", name="ones2_t")
            nc.vector.memset(ones2_t[:], 1.0)
            ones64_t = cpool.tile([64, 1], dt.bfloat16, tag="ones64", name="ones64_t")
            nc.vector.memset(ones64_t[:], 1.0)

            # persistent count accumulator: count[b, n] over this core's 64 b
            count = kpool.tile([64, N], dt.float32, tag="count", name="count")

            pending = []  # (t, mask_tile) whose count-matmul is deferred

            def flush_one():
                t, m = pending.pop(0)
                for h in range(2):
                    nc.tensor.matmul(
                        count[:, h * 512:(h + 1) * 512],
                        lhsT=bmap_t[:, 63 - 2 * t:127 - 2 * t],
                        rhs=m[:, h * 512:(h + 1) * 512],
                        start=(t == 0), stop=(t == NT - 1),
                    )

            for g in range(NT // GROUP):
                fgroup = fpool.tile([128, GROUP * 512], dt.bfloat16, tag="fg",
                                    name=f"fg{g}")
                nc.sync.dma_start(
                    out=fgroup[:],
                    in_=fT[:, g * GROUP * 512:(g + 1) * GROUP * 512])
                for j in range(GROUP):
                    t = g * GROUP + j
                    s = spool.tile([128, N], dt.float32, tag="s", name=f"s{t}")
                    for c in range(4):
                        lhsT = fgroup[:, j * 512 + c * 128: j * 512 + (c + 1) * 128]
                        for h in range(2):
                            nc.tensor.matmul(
                                s[:, h * 512:(h + 1) * 512],
                                lhsT=lhsT,
                                rhs=ct[c][:, h * 512:(h + 1) * 512],
                                start=(c == 0), stop=False,
                            )
                    # bias: s += ones2.T @ (hi/lo of -0.5*||c||^2)
                    for h in range(2):
                        nc.tensor.matmul(
                            s[:, h * 512:(h + 1) * 512],
                            lhsT=ones2_t[:],
                            rhs=csq2_t[:, h * 512:(h + 1) * 512],
                            start=False, stop=True,
                        )
                    mneg = mnpool.tile([128, 1], dt.float32, tag="mneg",
                                       name=f"mneg{t}")
                    nc.vector.reduce_max(out=mneg[:], in_=s[:],
                                         axis=mybir.AxisListType.X, negate=True)
                    mask = mpool.tile([128, N], dt.bfloat16, tag="mask",
                                      name=f"mask{t}")
                    nc.scalar.sign(out=mask[:], in_=s[:], bias=mneg[:])
                    pending.append((t, mask))
                    if len(pending) > COUNT_DELAY:
                        flush_one()
            while pending:
                flush_one()

            # coverage[b, n] = 1[count >= -63.5]; then reduce over b.
            covb = opool.tile([64, N], dt.bfloat16, tag="covb", name="covb")
            nc.vector.tensor_scalar(covb[:], count[:], -63.5, None,
                                    mybir.AluOpType.is_ge)
            tot = spool.tile([1, N], dt.float32, tag="s", name="tot")
            for h in range(2):
                nc.tensor.matmul(tot[:, h * 512:(h + 1) * 512],
                                 lhsT=ones64_t[:],
                                 rhs=covb[:, h * 512:(h + 1) * 512],
                                 start=True, stop=True)
            res = opool.tile([1, N], dt.float32, tag="res", name="res")
            nc.scalar.copy(out=res[:], in_=tot[:])
            nc.sync.dma_start(out=cov[:], in_=res[:])

    nc.compile()
    return nc


def _get_nc():
    if "nc" not in _CACHE:
        _CACHE["nc"] = _build_nc()
    return _CACHE["nc"]


def _prep_inputs(selected_frames: np.ndarray, cluster_centers: np.ndarray):
    frames = np.ascontiguousarray(np.asarray(selected_frames, dtype=np.float32))
    centers = np.ascontiguousarray(np.asarray(cluster_centers, dtype=np.float32))

    # centers^T, D-major, chunked into 4 partition blocks of 128.
    cT = np.ascontiguousarray(
        centers.T.reshape(4, 128, N).astype(BF16))

    # -0.5*||c||^2 split into bf16 hi + lo for fp32-accurate bias.
    v = (-0.5 * (centers.astype(np.float64) ** 2).sum(-1)).astype(np.float32)
    hi = v.astype(BF16)
    lo = (v - hi.astype(np.float32)).astype(BF16)
    csq2 = np.ascontiguousarray(np.stack([hi, lo]))  # [2, N] bf16

    # block-ones map: column window [63-2t, 127-2t) gives the [128, 64]
    # stationary matrix sending frame row f to batch row 2t + f//64.
    bmap = np.zeros((128, 127), dtype=BF16)
    bmap[0:64, 63] = 1
    bmap[64:128, 64] = 1

    in_maps = []
    fl = frames.reshape(B, K, D)
    for core in range(NCORES):
        fc = fl[core * B_PER_CORE:(core + 1) * B_PER_CORE].reshape(F_PER_CORE, D)
        # fT[d, t, c, f] = fc[128t+f, 128c+d]  -> [128, NT*512]
        fT = np.ascontiguousarray(
            fc.reshape(NT, 128, 4, 128).transpose(3, 0, 2, 1)
        ).reshape(128, NT * 512).astype(BF16)
        fT = np.ascontiguousarray(fT)
        in_maps.append({"fT": fT, "cT": cT, "csq2": csq2, "bmap": bmap})
    return in_maps


def kernel(selected_frames: np.ndarray, cluster_centers: np.ndarray) -> np.ndarray:
    global LAST_RESULTS
    nc = _get_nc()
    in_maps = _prep_inputs(selected_frames, cluster_centers)
    res = run_bass_kernel_spmd(nc, in_maps, list(range(NCORES)), **RUN_KWARGS)
    LAST_RESULTS = res
    cov = np.zeros(N, dtype=np.float64)
    for core in range(NCORES):
        cov += res.results[core]["cov"].reshape(N).astype(np.float64)
    prob = cov / (B * K)
    entropy = -(prob * np.log(prob + 1e-10)).sum()
    return np.float32(entropy)


if __name__ == "__main__":
    rng = np.random.default_rng(0)
    sf = rng.standard_normal((B, K, D), dtype=np.float32)
    cc = rng.standard_normal((N, D), dtype=np.float32)
    out = kernel(sf, cc)
    print("kernel out:", out)


# revision 16
# speedup vs baseline: 1.1530x; 1.1530x over previous
"""ClusterLoss (vq codebook coverage entropy) Trainium2 kernel.

Problem (hardcoded shapes):
    selected_frames [B=512, K=64, D=512] f32, cluster_centers [N=1024, D=512] f32.
    assignments = argmin_n ||x_{b,k} - c_n||^2
    coverage[n]  = sum_b 1[any_k assignments[b,k] == n]
    prob = coverage / (B*K);  out = -sum prob*log(prob+1e-10)   (scalar f32)

Strategy:
    * Data-parallel over batch: 64 batch elements (4096 frames) per core on 8
      cores; cluster_centers replicated.
    * argmin_n dist^2 == argmax_n s,  s = cross - 0.5*||c_n||^2 (x^2 is
      constant per row).  Per 128-frame tile: cross is 8 bf16 matmuls into
      fp32 PSUM (peak-rate); the bias add + row-max fuse into one VectorE
      tensor_tensor_reduce (s -> SBUF fp32, m = rowmax -> SBUF).
    * mask = sign(-s + m) in {0 (argmax), 1} via one ScalarE activation.
    * count[b, n] = sum_k mask[k, n] = 64 - #argmax-hits via a block-ones
      stationary matmul, accumulated over all 32 tiles in persistent PSUM.
    * count PSUM -> SBUF -> DRAM; host computes
      coverage[n] = sum_b 1[count <= 63.5], then prob/entropy, summing the 8
      per-core [64, 1024] count blocks.

bf16 effect on assignments measured off-line: ~127/32768 flipped assignments,
rel entropy error ~6.6e-4 (kernel matches its bf16 numpy model exactly).
"""

import numpy as np
import ml_dtypes

import concourse.bass as bass
import concourse.mybir as mybir
import concourse.tile as tile
from concourse import bacc
from concourse.bass_utils import run_bass_kernel_spmd

import os
BF16 = ml_dtypes.bfloat16
VARIANT = os.environ.get("KERNEL_VARIANT", "nottr")

B, K, D, N = 512, 64, 512, 1024
NCORES = 8
B_PER_CORE = B // NCORES          # 64
F_PER_CORE = B_PER_CORE * K       # 4096 frames
NT = F_PER_CORE // 128            # 32 tiles of 128 frames (2 batch elems)
GROUP = 8                         # f-tiles per frame-load group
SUB = 2                           # DMAs per group (split for earlier start)
COUNT_DELAY = 3                   # tiles to delay the count matmul (pipelining)
NEG_INF = -3.0e38

_CACHE = {}
LAST_RESULTS = None
RUN_KWARGS = {}  # extra kwargs for run_bass_kernel_spmd (profiling harness hook)


def _build_nc():
    nc = bacc.Bacc("TRN2", target_bir_lowering=False, debug=False,
                   num_devices=NCORES)
    dt = mybir.dt

    # DRAM I/O (per core).  fT layout: fT[d, t*512 + c*128 + f] =
    # frames[128*t + f, 128*c + d]  (pre-transposed on host: all matmul
    # operands are D-major).
    fT = nc.dram_tensor("fT", [128, NT * 512], dt.bfloat16, kind="ExternalInput")
    cT = nc.dram_tensor("cT", [4, 128, N], dt.bfloat16, kind="ExternalInput")
    bmap = nc.dram_tensor("bmap", [128, 127], dt.bfloat16, kind="ExternalInput")
    cnt_out = nc.dram_tensor("cnt", [64, N], dt.float32, kind="ExternalOutput")

    with tile.TileContext(nc) as tc:
        with (
            tc.tile_pool(name="const", bufs=1) as cpool,
            tc.tile_pool(name="frames", bufs=2) as fpool,
            tc.tile_pool(name="sbias", bufs=2) as sbpool,
            tc.tile_pool(name="mask", bufs=COUNT_DELAY + 2) as mpool,
            tc.tile_pool(name="mrow", bufs=4) as mnpool,
            tc.tile_pool(name="spsum", bufs=2, space="PSUM") as spool,
            tc.tile_pool(name="cpsum", bufs=1, space="PSUM") as kpool,
            tc.tile_pool(name="outp", bufs=1) as opool,
        ):
            # frame loads on SWDGE (gpsimd); consts on HWDGE run in parallel.
            SUBW = GROUP * 512 // SUB
            fgs = {}

            def load_group(g):
                for u in range(SUB):
                    fg = fpool.tile([128, SUBW], dt.bfloat16, tag=f"fg{u}",
                                    name=f"fg{g}_{u}")
                    fgs[(g, u)] = fg
                    nc.gpsimd.dma_start(
                        out=fg[:],
                        in_=fT[:, g * GROUP * 512 + u * SUBW:
                               g * GROUP * 512 + (u + 1) * SUBW])

            load_group(0)

            ct = []
            for c in range(4):
                t_ = cpool.tile([128, N], dt.bfloat16, tag=f"ct{c}", name=f"ct{c}")
                nc.sync.dma_start(out=t_[:], in_=cT[c])
                ct.append(t_)
            bmap_t = cpool.tile([128, 127], dt.bfloat16, tag="bmap", name="bmap_t")
            nc.sync.dma_start(out=bmap_t[:], in_=bmap[:])
            if VARIANT == "biasmm":
                csq2 = nc.dram_tensor("csq2", [2, N], dt.bfloat16,
                                      kind="ExternalInput")
                csq2_t = cpool.tile([2, N], dt.bfloat16, tag="csq2",
                                    name="csq2_t")
                nc.sync.dma_start(out=csq2_t[:], in_=csq2[:])
                ones2_t = cpool.tile([2, 128], dt.bfloat16, tag="ones2",
                                     name="ones2_t")
                nc.vector.memset(ones2_t[:], 1.0)
            else:
                csqb = nc.dram_tensor("csqb", [128, N], dt.float32,
                                      kind="ExternalInput")
                csqb_t = cpool.tile([128, N], dt.float32, tag="csqb",
                                    name="csqb_t")
                nc.sync.dma_start(out=csqb_t[:], in_=csqb[:])

            # persistent count accumulator: count[b, n] = 64 - #argmax-hits
            count = kpool.tile([64, N], dt.float32, tag="count", name="count")

            pending = []  # (t, mask_tile) with deferred count-matmul

            def flush_one():
                t, m = pending.pop(0)
                for h in range(2):
                    nc.tensor.matmul(
                        count[:, h * 512:(h + 1) * 512],
                        lhsT=bmap_t[:, 63 - 2 * t:127 - 2 * t],
                        rhs=m[:, h * 512:(h + 1) * 512],
                        start=(t == 0), stop=(t == NT - 1),
                    )

            for g in range(NT // GROUP):
                if g + 1 < NT // GROUP:
                    load_group(g + 1)  # double-buffered prefetch
                for j in range(GROUP):
                    t = g * GROUP + j
                    fg = fgs[(g, j // (GROUP // SUB))]
                    jj = j % (GROUP // SUB)
                    s = spool.tile([128, N], dt.float32, tag="s", name=f"s{t}")
                    last_chunk = 3 if VARIANT != "biasmm" else -1
                    for c in range(4):
                        lhsT = fg[:, jj * 512 + c * 128: jj * 512 + (c + 1) * 128]
                        for h in range(2):
                            nc.tensor.matmul(
                                s[:, h * 512:(h + 1) * 512],
                                lhsT=lhsT,
                                rhs=ct[c][:, h * 512:(h + 1) * 512],
                                start=(c == 0), stop=(c == last_chunk),
                            )
                    mrow = mnpool.tile([128, 1], dt.float32, tag="mrow",
                                       name=f"mrow{t}")
                    if VARIANT == "biasmm":
                        # bias via rank-2 matmul: s += ones2.T @ (hi/lo csq)
                        for h in range(2):
                            nc.tensor.matmul(
                                s[:, h * 512:(h + 1) * 512],
                                lhsT=ones2_t[:],
                                rhs=csq2_t[:, h * 512:(h + 1) * 512],
                                start=False, stop=True,
                            )
                        cmp_src = s
                        nc.vector.reduce_max(out=mrow[:], in_=s[:],
                                             axis=mybir.AxisListType.X)
                    else:
                        # sb = s + csqb (VectorE), m = rowmax(sb)
                        sb = sbpool.tile([128, N], dt.float32, tag="sb",
                                         name=f"sb{t}")
                        nc.vector.tensor_tensor(
                            out=sb[:], in0=s[:], in1=csqb_t[:],
                            op=mybir.AluOpType.add)
                        nc.vector.reduce_max(out=mrow[:], in_=sb[:],
                                             axis=mybir.AxisListType.X)
                        cmp_src = sb
                    # mask = sign(m - sb) in {0 (argmax), 1 (other)}
                    mask = mpool.tile([128, N], dt.bfloat16, tag="mask",
                                      name=f"mask{t}")
                    nc.scalar.activation(mask[:], cmp_src[:],
                                         mybir.ActivationFunctionType.Sign,
                                         bias=mrow[:], scale=-1.0)
                    pending.append((t, mask))
                    if len(pending) > COUNT_DELAY:
                        flush_one()
            while pending:
                flush_one()

            res = opool.tile([64, N], dt.float32, tag="res", name="res")
            nc.scalar.copy(out=res[:], in_=count[:])
            nc.sync.dma_start(out=cnt_out[:], in_=res[:])

    nc.compile()
    return nc


def _get_nc():
    if "nc" not in _CACHE:
        _CACHE["nc"] = _build_nc()
    return _CACHE["nc"]


def _prep_inputs(selected_frames: np.ndarray, cluster_centers: np.ndarray):
    frames = np.ascontiguousarray(np.asarray(selected_frames, dtype=np.float32))
    centers = np.ascontiguousarray(np.asarray(cluster_centers, dtype=np.float32))

    # centers^T, D-major, chunked into 4 partition blocks of 128.
    cT = np.ascontiguousarray(centers.T.reshape(4, 128, N).astype(BF16))

    # -0.5*||c||^2 (fp32), replicated across the 128 partitions (nottr),
    # plus a bf16 hi/lo split (biasmm).
    v = (-0.5 * (centers.astype(np.float64) ** 2).sum(-1)).astype(np.float32)
    csqb = np.ascontiguousarray(np.broadcast_to(v, (128, N)))
    hi = v.astype(BF16)
    lo = (v - hi.astype(np.float32)).astype(BF16)
    csq2 = np.ascontiguousarray(np.stack([hi, lo]))

    # block-ones map: column window [63-2t, 127-2t) is the [128, 64]
    # stationary matrix sending frame row f to batch row 2t + f//64.
    bmap = np.zeros((128, 127), dtype=BF16)
    bmap[0:64, 63] = 1
    bmap[64:128, 64] = 1

    in_maps = []
    fl = frames.reshape(B, K, D)
    for core in range(NCORES):
        fc = fl[core * B_PER_CORE:(core + 1) * B_PER_CORE].reshape(F_PER_CORE, D)
        # fT[d, t, c, f] = fc[128t+f, 128c+d]  -> [128, NT*512]
        fT = np.ascontiguousarray(
            fc.reshape(NT, 128, 4, 128).transpose(3, 0, 2, 1)
        ).reshape(128, NT * 512).astype(BF16)
        fT = np.ascontiguousarray(fT)
        in_maps.append({"fT": fT, "cT": cT, "csqb": csqb, "csq2": csq2,
                        "bmap": bmap})
    return in_maps


def kernel(selected_frames: np.ndarray, cluster_centers: np.ndarray) -> np.ndarray:
    global LAST_RESULTS
    nc = _get_nc()
    in_maps = _prep_inputs(selected_frames, cluster_centers)
    res = run_bass_kernel_spmd(nc, in_maps, list(range(NCORES)), **RUN_KWARGS)
    LAST_RESULTS = res
    cov = np.zeros(N, dtype=np.float64)
    for core in range(NCORES):
        cnt = res.results[core]["cnt"]  # [64, N], value 64 - #argmax-hits
        cov += (cnt <= 63.5).sum(axis=0)
    prob = cov / (B * K)
    entropy = -(prob * np.log(prob + 1e-10)).sum()
    return np.float32(entropy)


if __name__ == "__main__":
    rng = np.random.default_rng(0)
    sf = rng.standard_normal((B, K, D), dtype=np.float32)
    cc = rng.standard_normal((N, D), dtype=np.float32)
    out = kernel(sf, cc)
    print("kernel out:", out)
